# revision 8
# baseline (speedup 1.0000x reference)
"""Trainium2 Bass kernel for nn_NystromLinearKernel.

Math: the reference output is affine in the query coordinates:
  u[b,o,y,x] = A0[b,o]*xs[x] + A1[b,o]*ys[y] - const[b,o]
with (cnt = histogram of sampled flat indices over the HxW grid)
  svsum[b,c] = sum_{y,x} cnt[y,x] * v[b,c,y,x]
  Mx[b,c]    = sum_{y,x} cnt[y,x] * xs[x] * v[b,c,y,x]
  My[b,c]    = sum_{y,x} cnt[y,x] * ys[y] * v[b,c,y,x]
  A0[b,o]    = sum_c weight[o,c,0] * svsum[b,c]
  A1[b,o]    = sum_c weight[o,c,1] * svsum[b,c]
  const[b,o] = sum_c (weight[o,c,0]*Mx[b,c] + weight[o,c,1]*My[b,c])

Sharding: data parallel, 1 batch element per NeuronCore (8 cores).
"""

import numpy as np

import concourse.bass as bass
import concourse.bacc as bacc
import concourse.tile as tile
from concourse import mybir
from concourse.bass_utils import run_bass_kernel_spmd

B, C, O, H, W = 8, 32, 32, 128, 128
J = H * W
N_CORES = 8
FP = mybir.dt.float32

CH = 4            # channels per DMA/compute chunk
NCHUNK = C // CH  # 8
UF = 512          # free elems per u-synthesis chunk (1 psum bank)
NU = (H * W // 4) // UF * 4 // 4  # 8 chunks of (128, 512)

_nc_cache = {}
last_result = None  # BassKernelResults of the most recent run (for test harness)


def _build_nc():
    nc = bacc.Bacc()
    v_d = nc.declare_dram_parameter("v", [C, H, W], FP, isOutput=False)
    cnt_d = nc.declare_dram_parameter("cnt", [H, W], FP, isOutput=False)
    ycols_d = nc.declare_dram_parameter("ycols", [H, 2], FP, isOutput=False)
    xext_d = nc.declare_dram_parameter("xext", [W, 6], FP, isOutput=False)
    wrep_d = nc.declare_dram_parameter("wrep", [64, 128], FP, isOutput=False)
    mask6_d = nc.declare_dram_parameter("mask6", [6, 128], FP, isOutput=False)
    r_d = nc.declare_dram_parameter("rmat", [6, 4096], FP, isOutput=False)
    u_d = nc.declare_dram_parameter("u", [O, H, W], FP, isOutput=True)

    with tile.TileContext(nc) as tc:
        with (
            tc.tile_pool(name="singles", bufs=1) as singles,
            tc.tile_pool(name="ps_small", bufs=1, space="PSUM") as ps_small,
            tc.tile_pool(name="u_ps_pool", bufs=4, space="PSUM") as u_ps_pool,
            tc.tile_pool(name="u_sb_pool", bufs=4) as u_sb_pool,
        ):
            def single(shape, name):
                return singles.tile(shape, FP, name=name, tag=name)

            cnt_sb = single([H, W], "cnt_sb")
            ycols_sb = single([H, 2], "ycols_sb")
            xext_sb = single([W, 6], "xext_sb")
            wrep_sb = single([64, 128], "wrep_sb")
            mask6_sb = single([6, 128], "mask6_sb")
            r_sb = single([6, 4096], "r_sb")
            v_sb = single([H, C, W], "v_sb")      # (y, c, x)
            vt_sb = single([H, C, W], "vt_sb")    # cnt * v
            p1sb = single([128, 2, 32], "p1sb")   # (x, k, c)
            rhs6 = single([64, 6], "rhs6")
            l_sb = single([6, 128], "l_sb")

            p1_ps = ps_small.tile([128, 2, 32], FP, name="p1_ps", tag="p1_ps")
            out2_ps = ps_small.tile([64, 6], FP, name="out2_ps", tag="out2_ps")
            out6_ps = ps_small.tile([6, 128], FP, name="out6_ps", tag="out6_ps")

            # constants over SWDGE (gpsimd) so the sync ring is dedicated to v
            nc.gpsimd.dma_start(out=cnt_sb, in_=cnt_d[:])
            nc.gpsimd.dma_start(out=ycols_sb, in_=ycols_d[:])
            nc.gpsimd.dma_start(out=xext_sb, in_=xext_d[:])
            nc.gpsimd.dma_start(out=wrep_sb, in_=wrep_d[:])
            nc.gpsimd.dma_start(out=mask6_sb, in_=mask6_d[:])
            nc.gpsimd.dma_start(out=r_sb, in_=r_d[:])

            # rhs6 starts zeroed; three small copies fill the live entries
            nc.vector.memset(rhs6, 0.0)

            v_view = v_d[:].rearrange("c y x -> y c x")
            for cc in range(NCHUNK):
                sl = slice(cc * CH, (cc + 1) * CH)
                nc.sync.dma_start(out=v_sb[:, sl, :], in_=v_view[:, sl, :])
                cnt_b = bass.AP(
                    tensor=cnt_sb.tensor,
                    offset=cnt_sb.offset,
                    ap=[cnt_sb.ap[0], [0, CH], cnt_sb.ap[1]],
                )
                nc.vector.tensor_mul(
                    out=vt_sb[:, sl, :], in0=v_sb[:, sl, :], in1=cnt_b
                )
                for c in range(cc * CH, (cc + 1) * CH):
                    # out[x, k] = sum_y vt[y,c,x] * ycols[y,k]
                    nc.tensor.matmul(
                        out=p1_ps[:, :, c],
                        lhsT=vt_sb[:, c, :],
                        rhs=ycols_sb,
                        start=True,
                        stop=True,
                    )

            nc.vector.tensor_copy(out=p1sb, in_=p1_ps)
            p1flat = p1sb.rearrange("x k c -> x (k c)")

            # out2[k*32+c, 0:2] = [colsum, xs-weighted colsum]
            nc.tensor.matmul(
                out=out2_ps[:, 0:2], lhsT=p1flat, rhs=xext_sb[:, 0:2],
                start=True, stop=True,
            )
            # out2[32+c, 2:6] = svsum[c] replicated (ones columns)
            nc.tensor.matmul(
                out=out2_ps[32:64, 2:6], lhsT=p1sb[:, 0, :], rhs=xext_sb[:, 2:6],
                start=True, stop=True,
            )

            # rhs6[c, 0:2] = [svsum, Mx]; rhs6[32+c, 1] = My; rhs6[32+c, 2:6] = svsum
            nc.vector.tensor_copy(out=rhs6[0:32, 0:2], in_=out2_ps[0:32, 0:2])
            nc.vector.tensor_copy(out=rhs6[32:64, 1:2], in_=out2_ps[32:64, 0:1])
            nc.vector.tensor_copy(out=rhs6[32:64, 2:6], in_=out2_ps[32:64, 2:6])

            # out6[m, yb*32+o]: m=0 -> A0, m=1 -> const, m=2+q -> A1
            nc.tensor.matmul(
                out=out6_ps, lhsT=rhs6, rhs=wrep_sb, start=True, stop=True
            )
            # L = out6 * mask6  (row1 scaled by -1, rows 2+q masked to block q)
            nc.vector.tensor_mul(out=l_sb, in0=out6_ps, in1=mask6_sb)

            # u[p, f] = sum_m L[m, p] * R[m, f], p = yb*32+o, f = ylo*128+x
            u_view = u_d[:].rearrange("o (yb ylo) x -> yb o ylo x", yb=4)
            for j in range(NU):
                u_ps = u_ps_pool.tile([128, UF], FP, name=f"u_ps{j}", tag="u_ps")
                nc.tensor.matmul(
                    out=u_ps,
                    lhsT=l_sb,
                    rhs=r_sb[:, j * UF : (j + 1) * UF],
                    start=True,
                    stop=True,
                )
                u_sb = u_sb_pool.tile([128, UF], FP, name=f"u_sb{j}", tag="u_sb")
                nc.scalar.copy(out=u_sb, in_=u_ps)
                nc.scalar.dma_start(
                    out=u_view[:, :, j * 4 : (j + 1) * 4, :],
                    in_=u_sb.rearrange("p (ylo x) -> p ylo x", ylo=4),
                )

    return nc


def _get_nc():
    if "nc" not in _nc_cache:
        nc = _build_nc()
        if not nc.is_finalized():
            nc.finalize()
        _nc_cache["nc"] = nc
    return _nc_cache["nc"]


def make_host_constants(weight, indices):
    idx = np.asarray(indices).astype(np.int64).ravel()
    w = np.asarray(weight, dtype=np.float32)
    cnt = np.bincount(idx, minlength=J).astype(np.float32).reshape(H, W)
    xs = np.linspace(-1.0, 1.0, W, dtype=np.float32)
    ys = np.linspace(-1.0, 1.0, H, dtype=np.float32)
    ycols = np.stack([np.ones(H, np.float32), ys], axis=1)
    xext = np.concatenate(
        [np.stack([np.ones(W, np.float32), xs], axis=1), np.ones((W, 4), np.float32)],
        axis=1,
    )
    wk = w.transpose(2, 1, 0).reshape(64, O)  # [k*32+c, o] = weight[o,c,k]
    wrep = np.ascontiguousarray(np.tile(wk, (1, 4)))  # col yb*32+o
    mask6 = np.zeros((6, 128), np.float32)
    mask6[0, :] = 1.0
    mask6[1, :] = -1.0
    for q in range(4):
        mask6[2 + q, 32 * q : 32 * (q + 1)] = 1.0
    rmat = np.zeros((6, 4096), np.float32)
    rmat[0] = np.tile(xs, 32)
    rmat[1] = 1.0
    ylo = np.repeat(np.arange(32), 128)
    for q in range(4):
        rmat[2 + q] = ys[q * 32 + ylo]
    return {
        "cnt": cnt,
        "ycols": ycols,
        "xext": xext,
        "wrep": wrep,
        "mask6": mask6,
        "rmat": rmat,
    }


def kernel(v, weight, indices, trace=False):
    global last_result
    v = np.ascontiguousarray(np.asarray(v, dtype=np.float32))
    consts = make_host_constants(weight, indices)
    nc = _get_nc()
    in_maps = [{"v": v[i], **consts} for i in range(N_CORES)]
    res = run_bass_kernel_spmd(nc, in_maps, list(range(N_CORES)), trace=trace)
    last_result = res
    u = np.stack([res.results[i]["u"] for i in range(N_CORES)], axis=0)
    return u
